# revision 35
# baseline (speedup 1.0000x reference)
"""Distributed Trainium2 kernel for 16-head causal attention (B=4, T=2048, D=1024).

Sharding (Megatron-style, per the hint): 8 cores = 4 batch pairs.
Core c handles batch c//2 and head-group c%2 (8 heads = 512 of D).
Each core computes its QKV projections (transposed layout), causal
attention for its 8 heads (scores computed as S^T = K Q^T so the AV
matmul needs no transposes; softmax needs no max-subtraction since
scores are ~N(0,1); the denominator comes for free from a ones-column
appended to V), then its partial output projection.  The two cores of a
batch pair combine bf16 partials with pairwise ReduceScatters (two
chunks, overlapping the output projection); the host concatenates the
row-quarters.

Performance structure:
- i-chunk-outer loop interleaves attention, softmax normalization and
  the output projection so the TensorEngine always has independent
  matmul work and stays HAM-warm.
- PSUM tiles are 2 banks ([128, 1024] f32) so exp / PSUM->SBUF copies
  cover 1024 columns per instruction (the ACT engine has a ~293ns
  fixed cost per instruction).
- Softmax normalization is deferred off the AV critical path: the AV
  matmul emits unnormalized attn^T plus a denominator row (from the
  ones-column), normalization happens per i-chunk with a batched
  reciprocal_approx_fast + DRAM-broadcast DMAs + in-place multiplies.
"""

import sys

sys.path.insert(0, "/opt/trn_rl_repo")

import numpy as np
import ml_dtypes

import concourse.bass as bass
import concourse.mybir as mybir
import concourse.tile as tile
from concourse import bacc
from concourse.bass_utils import run_bass_kernel_spmd

BF16 = mybir.dt.bfloat16
F32 = mybir.dt.float32
P = 128
D_MODEL = 1024
D_LOCAL = 512  # 8 heads x 64 per core
H_LOCAL = 8
HD = 64
N_CORES = 8
EXP_SCALE = 0.125  # 1/sqrt(64)
NCH = 4  # ReduceScatter chunks

Exp = mybir.ActivationFunctionType.Exp
Mult = mybir.AluOpType.mult


def build_nc(T, debug_taps=False):
    """Build the SPMD Bass graph (identical on all 8 cores)."""
    assert T % 512 == 0
    TB = T // 128  # t-blocks
    TC = T // 512  # i-chunks

    nc = bacc.Bacc(None, target_bir_lowering=False, debug=False,
                   num_devices=N_CORES)

    xT_d = nc.dram_tensor("xT", [D_MODEL, T], BF16, kind="ExternalInput")
    wqT_d = nc.dram_tensor("wqT", [D_MODEL, D_LOCAL], BF16, kind="ExternalInput")
    wkT_d = nc.dram_tensor("wkT", [D_MODEL, D_LOCAL], BF16, kind="ExternalInput")
    wvT_d = nc.dram_tensor("wvT", [D_MODEL, D_LOCAL], BF16, kind="ExternalInput")
    woT_d = nc.dram_tensor("woT", [D_LOCAL, D_MODEL], BF16, kind="ExternalInput")
    out_d = nc.dram_tensor("out", [T // 2, D_MODEL], F32, kind="ExternalOutput")

    # chunked pairwise ReduceScatter buffers (bf16)
    rs_in = [nc.dram_tensor(f"rs_in{c}", [T // NCH, D_MODEL], BF16)
             for c in range(NCH)]
    rs_out = [nc.dram_tensor(f"rs_out{c}", [T // (2 * NCH), D_MODEL], BF16)
              for c in range(NCH)]

    # Upper-triangular (incl. diagonal) multiplicative mask for the
    # transposed-score layout: e^T[j, i] valid iff i >= j.
    tri_np = (np.arange(128)[None, :] >= np.arange(128)[:, None])
    tri_d = nc.inline_tensor(tri_np.astype(ml_dtypes.bfloat16), name="tri")
    ones_d = nc.inline_tensor(np.ones((P, P), dtype=ml_dtypes.bfloat16),
                              name="onesblk")

    with tile.TileContext(nc) as tc:
        with (
            tc.tile_pool(name="persist", bufs=1) as wpool,
            tc.tile_pool(name="efull", bufs=8) as epool,
            tc.tile_pool(name="ediag", bufs=6) as edpool,
            tc.tile_pool(name="small", bufs=2) as spool,
            tc.tile_pool(name="osb", bufs=3) as opool,
            tc.tile_pool(name="dscratch", bufs=2, space="DRAM") as dpool,
            tc.tile_pool(name="psum", bufs=3, space="PSUM") as psum,
            tc.tile_pool(name="psum_av", bufs=2, space="PSUM") as psum_av,
        ):
            tri_sb = wpool.tile([P, P], BF16, tag="tri")
            nc.sync.dma_start(tri_sb[:], tri_d.ap())
            ones_sb = wpool.tile([P, P], BF16, tag="ones")
            nc.sync.dma_start(ones_sb[:], ones_d.ap())

            xT_sb = wpool.tile([P, 8, T], BF16, tag="xT")
            wq_sb = wpool.tile([P, 8, D_LOCAL], BF16, tag="wq")
            wk_sb = wpool.tile([P, 8, D_LOCAL], BF16, tag="wk")
            wv_sb = wpool.tile([P, 8, D_LOCAL], BF16, tag="wv")
            wo_sb = wpool.tile([P, 4, D_MODEL], BF16, tag="wo")
            qT_sb = wpool.tile([P, 4, T], BF16, tag="qT")
            kT_sb = wpool.tile([P, 4, T], BF16, tag="kT")
            # v with a ones-column appended per head (65 cols per head)
            v_sb = wpool.tile([P, TB, H_LOCAL * 65], BF16, tag="v")
            attnT_sb = wpool.tile([P, 4, T], BF16, tag="attnT")

            wq_r = wqT_d.ap().rearrange("(o p) d -> o p d", p=P)
            wk_r = wkT_d.ap().rearrange("(o p) d -> o p d", p=P)
            wv_r = wvT_d.ap().rearrange("(o p) d -> o p d", p=P)
            wo_r = woT_d.ap().rearrange("(o p) e -> o p e", p=P)
            xT_r = xT_d.ap().rearrange("(o p) t -> o p t", p=P)
            for o in range(8):
                nc.sync.dma_start(wv_sb[:, o], wv_r[o])
            for t0 in range(0, T, 512):
                for o in range(8):
                    nc.sync.dma_start(xT_sb[:, o, t0:t0 + 512],
                                      xT_r[o][:, t0:t0 + 512])
                if t0 == 0:
                    for o in range(8):
                        nc.sync.dma_start(wq_sb[:, o], wq_r[o])
            for o in range(8):
                nc.sync.dma_start(wk_sb[:, o], wk_r[o])
            for o in range(4):
                nc.sync.dma_start(wo_sb[:, o], wo_r[o])

            # ones columns of v (col 64 of each head's 65-wide slot):
            # one strided DVE copy from a dense const block
            v_view = v_sb[:].rearrange("p t (h c) -> p t h c", c=65)
            nc.vector.tensor_copy(
                v_view[:, :, :, 64:65],
                ones_sb[:, 0:TB * H_LOCAL].rearrange(
                    "p (t h o) -> p t h o", h=H_LOCAL, o=1),
            )

            # ---- projection emitters (interleaved into the chunk loop) ----
            def emit_qkproj(m):
                # q^T, k^T block m: [d, t] layout (lhsT = W^T, rhs = x^T)
                for w_sb, dst in ((wq_sb, qT_sb), (wk_sb, kT_sb)):
                    for t0 in range(0, T, 1024):
                        wdt = min(1024, T - t0)
                        ps = psum.tile([P, 1024], F32, tag="mm2")
                        for k in range(8):
                            for half in range(wdt // 512):
                                hs = slice(half * 512, half * 512 + 512)
                                nc.tensor.matmul(
                                    ps[:, hs],
                                    lhsT=w_sb[:, k, m * 128:(m + 1) * 128],
                                    rhs=xT_sb[:, k, t0 + half * 512:
                                              t0 + half * 512 + 512],
                                    start=(k == 0), stop=(k == 7),
                                )
                        nc.vector.tensor_copy(dst[:, m, t0:t0 + wdt],
                                              ps[:, 0:wdt])

            def emit_vproj(tb_lo, tb_hi):
                # v blocks: [t, d] layout (lhsT = x^T, rhs = W^T), scattered
                # into the 65-stride per-head slots; 2 t-blocks per psum
                for tb0 in range(tb_lo, tb_hi, 2):
                    ps = psum.tile([P, 1024], F32, tag="mm2")
                    for half in range(2):
                        tb = tb0 + half
                        hs = slice(half * 512, half * 512 + 512)
                        for k in range(8):
                            nc.tensor.matmul(
                                ps[:, hs],
                                lhsT=xT_sb[:, k, tb * 128:(tb + 1) * 128],
                                rhs=wv_sb[:, k, :],
                                start=(k == 0), stop=(k == 7),
                            )
                    nc.vector.tensor_copy(
                        v_view[:, tb0:tb0 + 2, :, 0:64],
                        ps[:].rearrange("p (t h c) -> p t h c", t=2, c=64),
                    )

            # ---- deferred per-chunk tail: normalize + out-proj + RS ----
            pending = []

            def emit_chunk_tail(ic):
                # out-projection for this chunk's 4 i-blocks (bf16 partials)
                for ib in range(4 * ic, 4 * ic + 4):
                    ch = min(ib // (TB // NCH), NCH - 1)
                    rbase = ib * 128 - ch * (TB // NCH) * 128
                    ps = psum.tile([P, 1024], F32, tag="mm2")
                    for dm in range(4):
                        for half in range(2):
                            hs = slice(half * 512, half * 512 + 512)
                            nc.tensor.matmul(
                                ps[:, hs],
                                lhsT=attnT_sb[:, dm, ib * 128:(ib + 1) * 128],
                                rhs=wo_sb[:, dm, half * 512:half * 512 + 512],
                                start=(dm == 0), stop=(dm == 3),
                            )
                    o = opool.tile([P, 1024], BF16, tag="o")
                    nc.vector.tensor_copy(o[:], ps[:])
                    nc.sync.dma_start(
                        rs_in[ch].ap()[rbase:rbase + 128, :], o[:])

                # fire each ReduceScatter chunk as soon as its rows exist
                for c in range(NCH):
                    last_ib = (c + 1) * (TB // NCH) - 1
                    if last_ib // 4 != ic:
                        continue
                    nc.gpsimd.collective_compute(
                        "ReduceScatter",
                        mybir.AluOpType.add,
                        replica_groups=[[0, 1], [2, 3], [4, 5], [6, 7]],
                        ins=[rs_in[c].ap().opt()],
                        outs=[rs_out[c].ap().opt()],
                    )

            # ---- per-(ic, m) attention emitter ----
            def emit_attn(ic, m, den):
                i0 = ic * 512
                nfull = i0 // 128
                if True:
                    e_full = {}  # (h_loc, jbp) -> [128, 1024] (jb pair)
                    e_d1 = {}    # h_loc -> [128, 896]: r=0 (512) | r=1 (384)
                    e_d2 = {}    # h_loc -> [128, 384]: r=2 (256) | r=3 (128)
                    rows_of = (slice(0, 64), slice(64, 128))
                    # full tiles: S^T = K Q^T, exp -> bf16 (no max needed);
                    # 2 j-blocks per psum tile / exp instruction
                    for jbp in range(nfull // 2):
                        pss = [psum.tile([P, 1024], F32, tag="mm2",
                                         name=f"qk{hl}") for hl in range(2)]
                        for half in range(2):
                            jb = 2 * jbp + half
                            hs = slice(half * 512, half * 512 + 512)
                            for h_loc in (0, 1):  # adjacent => row-packed
                                nc.tensor.matmul(
                                    pss[h_loc][:, hs],
                                    lhsT=kT_sb[rows_of[h_loc], m,
                                               jb * 128:(jb + 1) * 128],
                                    rhs=qT_sb[rows_of[h_loc], m, i0:i0 + 512],
                                    start=True, stop=True,
                                )
                        for h_loc in (0, 1):
                            e = epool.tile([P, 1024], BF16, tag="ef2")
                            nc.scalar.activation(e[:], pss[h_loc][:], Exp,
                                                 scale=EXP_SCALE)
                            e_full[(h_loc, jbp)] = e
                    # diagonal region: j-block nfull+r covers i-cols
                    # [r*128, 512) of the chunk in ONE matmul; r in {0,1}
                    # packed into one 2-bank psum (widths 512+384), r in
                    # {2,3} into one bank (256+128); the leading 128 cols
                    # of each r (s==r) get the triangular mask
                    for h_loc in (0, 1):
                        rows = rows_of[h_loc]
                        ps1 = psum.tile([P, 1024], F32, tag="mm2")
                        ps2 = psum.tile([P, 1024], F32, tag="mm2")
                        for r, ps, off in ((0, ps1, 0), (1, ps1, 512),
                                           (2, ps2, 0), (3, ps2, 256)):
                            jb = nfull + r
                            width = (4 - r) * 128
                            nc.tensor.matmul(
                                ps[:, off:off + width],
                                lhsT=kT_sb[rows, m, jb * 128:(jb + 1) * 128],
                                rhs=qT_sb[rows, m, i0 + r * 128:i0 + 512],
                                # off 0 / 512 land at a fresh psum bank: the
                                # first write there must set start (pends
                                # that 2KB zero-region); off 256 reuses r=2's
                                start=(off in (0, 512)), stop=True,
                                skip_group_check=True,
                            )
                        ed1 = edpool.tile([P, 896], BF16, tag="ed1")
                        nc.scalar.activation(ed1[:], ps1[:, 0:896], Exp,
                                             scale=EXP_SCALE)
                        ed2 = edpool.tile([P, 384], BF16, tag="ed2")
                        nc.scalar.activation(ed2[:], ps2[:, 0:384], Exp,
                                             scale=EXP_SCALE)
                        for ed, off in ((ed1, 0), (ed1, 512),
                                        (ed2, 0), (ed2, 256)):
                            nc.vector.tensor_tensor(
                                ed[:, off:off + 128], ed[:, off:off + 128],
                                tri_sb[:], Mult)
                        e_d1[h_loc] = ed1
                        e_d2[h_loc] = ed2
                    # AV: psum[0:64] = unnormalized attn^T, psum[64] = denom
                    for h_loc in (0, 1):
                        h = 2 * m + h_loc
                        vslot = slice(h * 65, (h + 1) * 65)
                        avps = psum_av.tile([P, 512], F32, tag="av")
                        for jbp in range(nfull // 2):
                            ef = e_full[(h_loc, jbp)]
                            for half in range(2):
                                jb = 2 * jbp + half
                                nc.tensor.matmul(
                                    avps[0:65, :],
                                    lhsT=v_sb[:, jb, vslot],
                                    rhs=ef[:, half * 512:half * 512 + 512],
                                    start=(jb == 0), stop=False,
                                    skip_group_check=True,
                                )
                        dslice = {0: (e_d1, 0), 1: (e_d1, 512),
                                  2: (e_d2, 0), 3: (e_d2, 256)}
                        for r in range(4):
                            edd, base = dslice[r]
                            ed = edd[h_loc]
                            width = (4 - r) * 128
                            nc.tensor.matmul(
                                avps[0:65, r * 128:512],
                                lhsT=v_sb[:, nfull + r, vslot],
                                rhs=ed[:, base:base + width],
                                # start=True pends the WHOLE psum bank
                                # (2KB zero-region): only the tile's very
                                # first matmul may set it
                                start=(nfull == 0 and r == 0),
                                stop=(r == 3),
                                skip_group_check=True,
                            )
                        # stash denominator row + unnormalized attn^T
                        # (DVE operands may sit at different partition bases)
                        nc.vector.tensor_copy(
                            den[0:1, h_loc * 512:h_loc * 512 + 512],
                            avps[64:65, :])
                        nc.vector.tensor_copy(
                            attnT_sb[h_loc * 64:h_loc * 64 + 64, m,
                                     i0:i0 + 512],
                            avps[0:64, :])

                # per-(ic, m) softmax normalization: reciprocal of the two
                # denominator rows, DRAM round-trip partition-broadcast,
                # one in-place multiply over both heads
                rec = spool.tile([P, 1024], F32, tag="rec")
                nc.vector.reciprocal_approx_fast(rec[0:1, :], den[0:1, :])
                recb = spool.tile([P, 1024], BF16, tag="recb")
                nc.vector.tensor_copy(recb[0:1, :], rec[0:1, :])
                rd = dpool.tile([1, 1024], BF16, tag="rd")
                nc.sync.dma_start(rd[:], recb[0:1, :])
                rb = spool.tile([P, 512], BF16, tag="rb")
                nc.sync.dma_start(rb[0:64, :],
                                  rd[0:1, 0:512].to_broadcast((64, 512)))
                nc.sync.dma_start(rb[64:128, :],
                                  rd[0:1, 512:1024].to_broadcast((64, 512)))
                nc.vector.tensor_tensor(
                    attnT_sb[:, m, i0:i0 + 512],
                    attnT_sb[:, m, i0:i0 + 512], rb[:], Mult)

            # ---- chunk schedule: interleave projections, attention and
            # deferred chunk-tails so PE always has independent matmuls ----
            emit_vproj(0, min(4, TB))
            for m in range(4):
                emit_qkproj(m)
                den = spool.tile([P, 1024], F32, tag="den")
                emit_attn(0, m, den)
            pending.append(0)
            for ic in range(1, TC):
                emit_vproj(4 * ic, min(4 * ic + 4, TB))
                for m in range(4):
                    den = spool.tile([P, 1024], F32, tag="den")
                    emit_attn(ic, m, den)
                    if m == 1 and pending:
                        # previous chunk's out-proj/RS: emitted mid-attention
                        # so its latency hides behind this chunk's QK/AV
                        emit_chunk_tail(pending.pop(0))
                pending.append(ic)

            while pending:
                emit_chunk_tail(pending.pop(0))

            qrt = T // (2 * NCH)  # rows per core per chunk
            for c in range(NCH):
                for blk in range(max(1, qrt // 128)):
                    rows = min(128, qrt)
                    cv = opool.tile([P, D_MODEL], BF16, tag="cv")
                    nc.gpsimd.dma_start(
                        cv[0:rows, :],
                        rs_out[c].ap()[blk * 128:blk * 128 + rows, :])
                    cf = opool.tile([P, D_MODEL], F32, tag="cf")
                    nc.vector.tensor_copy(cf[0:rows, :], cv[0:rows, :])
                    nc.gpsimd.dma_start(
                        out_d.ap()[c * qrt + blk * 128:
                                   c * qrt + blk * 128 + rows, :],
                        cf[0:rows, :])

            if debug_taps:
                qT_t = nc.dram_tensor("dbg_qT", [P, 4, T], BF16)
                kT_t = nc.dram_tensor("dbg_kT", [P, 4, T], BF16)
                v_t = nc.dram_tensor("dbg_v", [P, TB, H_LOCAL * 65], BF16)
                at_t = nc.dram_tensor("dbg_attnT", [P, 4, T], BF16)
                nc.sync.dma_start(qT_t.ap(), qT_sb[:])
                nc.sync.dma_start(kT_t.ap(), kT_sb[:])
                nc.sync.dma_start(v_t.ap(), v_sb[:])
                nc.sync.dma_start(at_t.ap(), attnT_sb[:])

    nc.finalize()  # Bacc: runs dce/alloc_regs/codegen passes
    return nc


_NC_CACHE = {}


def _get_nc(T):
    if T not in _NC_CACHE:
        _NC_CACHE[T] = build_nc(T)
    return _NC_CACHE[T]


def make_in_maps(x, Wq, Wk, Wv, Wo):
    bf = ml_dtypes.bfloat16
    in_maps = []
    for c in range(N_CORES):
        b, g = divmod(c, 2)
        gs = slice(g * D_LOCAL, (g + 1) * D_LOCAL)
        in_maps.append({
            "xT": np.ascontiguousarray(x[b].T).astype(bf),
            "wqT": np.ascontiguousarray(Wq[gs, :].T).astype(bf),
            "wkT": np.ascontiguousarray(Wk[gs, :].T).astype(bf),
            "wvT": np.ascontiguousarray(Wv[gs, :].T).astype(bf),
            "woT": np.ascontiguousarray(Wo[:, gs].T).astype(bf),
        })
    return in_maps


def assemble_out(outs, B, T, D):
    """Stitch per-core [T//2, D] chunked-RS outputs into [B, T, D]."""
    y = np.empty((B, T, D), np.float32)
    qrt = T // (2 * NCH)
    for b in range(B):
        ev, od = outs[2 * b]["out"], outs[2 * b + 1]["out"]
        for c in range(NCH):
            base = c * (T // NCH)
            y[b, base:base + qrt] = ev[c * qrt:(c + 1) * qrt]
            y[b, base + qrt:base + 2 * qrt] = od[c * qrt:(c + 1) * qrt]
    return y


# test harness hook: set RUN_OPTS["trace"]=True before calling kernel() to
# capture an NTFF profile; the BassKernelResults lands in RUN_OPTS["last"].
RUN_OPTS = {"trace": False, "tmpdir": None, "last": None}


def kernel(x, Wq, Wk, Wv, Wo):
    x = np.asarray(x, dtype=np.float32)
    B, T, D = x.shape
    nc = _get_nc(T)
    in_maps = make_in_maps(np.asarray(x), np.asarray(Wq), np.asarray(Wk),
                           np.asarray(Wv), np.asarray(Wo))
    res = run_bass_kernel_spmd(
        nc, in_maps, core_ids=list(range(N_CORES)),
        trace=RUN_OPTS["trace"], tmpdir=RUN_OPTS["tmpdir"],
    )
    RUN_OPTS["last"] = res
    return assemble_out(res.results, B, T, D)


# revision 36
# speedup vs baseline: 1.0239x; 1.0239x over previous
"""Distributed Trainium2 kernel for 16-head causal attention (B=4, T=2048, D=1024).

Sharding (Megatron-style, per the hint): 8 cores = 4 batch pairs.
Core c handles batch c//2 and head-group c%2 (8 heads = 512 of D).
Each core computes its QKV projections (transposed layout), causal
attention for its 8 heads (scores computed as S^T = K Q^T so the AV
matmul needs no transposes; softmax needs no max-subtraction since
scores are ~N(0,1); the denominator comes for free from a ones-column
appended to V), then its partial output projection.  The two cores of a
batch pair combine bf16 partials with pairwise ReduceScatters (two
chunks, overlapping the output projection); the host concatenates the
row-quarters.

Performance structure:
- i-chunk-outer loop interleaves attention, softmax normalization and
  the output projection so the TensorEngine always has independent
  matmul work and stays HAM-warm.
- PSUM tiles are 2 banks ([128, 1024] f32) so exp / PSUM->SBUF copies
  cover 1024 columns per instruction (the ACT engine has a ~293ns
  fixed cost per instruction).
- Softmax normalization is deferred off the AV critical path: the AV
  matmul emits unnormalized attn^T plus a denominator row (from the
  ones-column), normalization happens per i-chunk with a batched
  reciprocal_approx_fast + DRAM-broadcast DMAs + in-place multiplies.
"""

import sys

sys.path.insert(0, "/opt/trn_rl_repo")

import numpy as np
import ml_dtypes

import concourse.bass as bass
import concourse.mybir as mybir
import concourse.tile as tile
from concourse import bacc
from concourse.bass_utils import run_bass_kernel_spmd

BF16 = mybir.dt.bfloat16
F32 = mybir.dt.float32
P = 128
D_MODEL = 1024
D_LOCAL = 512  # 8 heads x 64 per core
H_LOCAL = 8
HD = 64
N_CORES = 8
EXP_SCALE = 0.125  # 1/sqrt(64)
NCH = 4  # ReduceScatter chunks

Exp = mybir.ActivationFunctionType.Exp
Mult = mybir.AluOpType.mult


def build_nc(T, debug_taps=False):
    """Build the SPMD Bass graph (identical on all 8 cores)."""
    assert T % 512 == 0
    TB = T // 128  # t-blocks
    TC = T // 512  # i-chunks

    nc = bacc.Bacc(None, target_bir_lowering=False, debug=False,
                   num_devices=N_CORES)

    xT_d = nc.dram_tensor("xT", [D_MODEL, T], BF16, kind="ExternalInput")
    wqT_d = nc.dram_tensor("wqT", [D_MODEL, D_LOCAL], BF16, kind="ExternalInput")
    wkT_d = nc.dram_tensor("wkT", [D_MODEL, D_LOCAL], BF16, kind="ExternalInput")
    wvT_d = nc.dram_tensor("wvT", [D_MODEL, D_LOCAL], BF16, kind="ExternalInput")
    woT_d = nc.dram_tensor("woT", [D_LOCAL, D_MODEL], BF16, kind="ExternalInput")
    out_d = nc.dram_tensor("out", [T // 2, D_MODEL], F32, kind="ExternalOutput")

    # chunked pairwise ReduceScatter buffers (bf16)
    rs_in = [nc.dram_tensor(f"rs_in{c}", [T // NCH, D_MODEL], BF16)
             for c in range(NCH)]
    rs_out = [nc.dram_tensor(f"rs_out{c}", [T // (2 * NCH), D_MODEL], BF16)
              for c in range(NCH)]

    # Upper-triangular (incl. diagonal) multiplicative mask for the
    # transposed-score layout: e^T[j, i] valid iff i >= j.
    tri_np = (np.arange(128)[None, :] >= np.arange(128)[:, None])
    tri_d = nc.inline_tensor(tri_np.astype(ml_dtypes.bfloat16), name="tri")
    ones_d = nc.inline_tensor(np.ones((P, P), dtype=ml_dtypes.bfloat16),
                              name="onesblk")

    with tile.TileContext(nc) as tc:
        with (
            tc.tile_pool(name="persist", bufs=1) as wpool,
            tc.tile_pool(name="efull", bufs=6) as epool,
            tc.tile_pool(name="ediag", bufs=4) as edpool,
            tc.tile_pool(name="small", bufs=3) as spool,
            tc.tile_pool(name="osb", bufs=3) as opool,
            tc.tile_pool(name="dscratch", bufs=2, space="DRAM") as dpool,
            tc.tile_pool(name="psum", bufs=3, space="PSUM") as psum,
            tc.tile_pool(name="psum_av", bufs=2, space="PSUM") as psum_av,
        ):
            tri_sb = wpool.tile([P, P], BF16, tag="tri")
            nc.sync.dma_start(tri_sb[:], tri_d.ap())
            ones_sb = wpool.tile([P, P], BF16, tag="ones")
            nc.sync.dma_start(ones_sb[:], ones_d.ap())

            xT_sb = wpool.tile([P, 8, T], BF16, tag="xT")
            wq_sb = wpool.tile([P, 8, D_LOCAL], BF16, tag="wq")
            wk_sb = wpool.tile([P, 8, D_LOCAL], BF16, tag="wk")
            wv_sb = wpool.tile([P, 8, D_LOCAL], BF16, tag="wv")
            wo_sb = wpool.tile([P, 4, D_MODEL], BF16, tag="wo")
            qT_sb = wpool.tile([P, 4, T], BF16, tag="qT")
            kT_sb = wpool.tile([P, 4, T], BF16, tag="kT")
            # v with a ones-column appended per head (65 cols per head)
            v_sb = wpool.tile([P, TB, H_LOCAL * 65], BF16, tag="v")
            attnT_sb = wpool.tile([P, 4, T], BF16, tag="attnT")

            wq_r = wqT_d.ap().rearrange("(o p) d -> o p d", p=P)
            wk_r = wkT_d.ap().rearrange("(o p) d -> o p d", p=P)
            wv_r = wvT_d.ap().rearrange("(o p) d -> o p d", p=P)
            wo_r = woT_d.ap().rearrange("(o p) e -> o p e", p=P)
            xT_r = xT_d.ap().rearrange("(o p) t -> o p t", p=P)
            for o in range(8):
                nc.sync.dma_start(wv_sb[:, o], wv_r[o])
            for t0 in range(0, T, 512):
                for o in range(8):
                    nc.sync.dma_start(xT_sb[:, o, t0:t0 + 512],
                                      xT_r[o][:, t0:t0 + 512])
                if t0 == 0:
                    for o in range(8):
                        nc.sync.dma_start(wq_sb[:, o], wq_r[o])
            for o in range(8):
                nc.sync.dma_start(wk_sb[:, o], wk_r[o])
            for o in range(4):
                nc.sync.dma_start(wo_sb[:, o], wo_r[o])

            # ones columns of v (col 64 of each head's 65-wide slot):
            # one strided DVE copy from a dense const block
            v_view = v_sb[:].rearrange("p t (h c) -> p t h c", c=65)
            nc.vector.tensor_copy(
                v_view[:, :, :, 64:65],
                ones_sb[:, 0:TB * H_LOCAL].rearrange(
                    "p (t h o) -> p t h o", h=H_LOCAL, o=1),
            )

            # ---- projection emitters (interleaved into the chunk loop) ----
            def emit_qkproj(m):
                # q^T, k^T block m: [d, t] layout (lhsT = W^T, rhs = x^T)
                for w_sb, dst in ((wq_sb, qT_sb), (wk_sb, kT_sb)):
                    for t0 in range(0, T, 1024):
                        wdt = min(1024, T - t0)
                        ps = psum.tile([P, 1024], F32, tag="mm2")
                        for k in range(8):
                            for half in range(wdt // 512):
                                hs = slice(half * 512, half * 512 + 512)
                                nc.tensor.matmul(
                                    ps[:, hs],
                                    lhsT=w_sb[:, k, m * 128:(m + 1) * 128],
                                    rhs=xT_sb[:, k, t0 + half * 512:
                                              t0 + half * 512 + 512],
                                    start=(k == 0), stop=(k == 7),
                                )
                        nc.vector.tensor_copy(dst[:, m, t0:t0 + wdt],
                                              ps[:, 0:wdt])

            def emit_vproj(tb_lo, tb_hi):
                # v blocks: [t, d] layout (lhsT = x^T, rhs = W^T), scattered
                # into the 65-stride per-head slots; 2 t-blocks per psum
                for tb0 in range(tb_lo, tb_hi, 2):
                    ps = psum.tile([P, 1024], F32, tag="mm2")
                    for half in range(2):
                        tb = tb0 + half
                        hs = slice(half * 512, half * 512 + 512)
                        for k in range(8):
                            nc.tensor.matmul(
                                ps[:, hs],
                                lhsT=xT_sb[:, k, tb * 128:(tb + 1) * 128],
                                rhs=wv_sb[:, k, :],
                                start=(k == 0), stop=(k == 7),
                            )
                    nc.vector.tensor_copy(
                        v_view[:, tb0:tb0 + 2, :, 0:64],
                        ps[:].rearrange("p (t h c) -> p t h c", t=2, c=64),
                    )

            # ---- deferred per-chunk tail: normalize + out-proj + RS ----
            pending = []

            def emit_chunk_tail(ic):
                # out-projection for this chunk's 4 i-blocks (bf16 partials)
                for ib in range(4 * ic, 4 * ic + 4):
                    ch = min(ib // (TB // NCH), NCH - 1)
                    rbase = ib * 128 - ch * (TB // NCH) * 128
                    ps = psum.tile([P, 1024], F32, tag="mm2")
                    for dm in range(4):
                        for half in range(2):
                            hs = slice(half * 512, half * 512 + 512)
                            nc.tensor.matmul(
                                ps[:, hs],
                                lhsT=attnT_sb[:, dm, ib * 128:(ib + 1) * 128],
                                rhs=wo_sb[:, dm, half * 512:half * 512 + 512],
                                start=(dm == 0), stop=(dm == 3),
                            )
                    o = opool.tile([P, 1024], BF16, tag="o")
                    nc.vector.tensor_copy(o[:], ps[:])
                    nc.sync.dma_start(
                        rs_in[ch].ap()[rbase:rbase + 128, :], o[:])

                # fire each ReduceScatter chunk as soon as its rows exist
                for c in range(NCH):
                    last_ib = (c + 1) * (TB // NCH) - 1
                    if last_ib // 4 != ic:
                        continue
                    nc.gpsimd.collective_compute(
                        "ReduceScatter",
                        mybir.AluOpType.add,
                        replica_groups=[[0, 1], [2, 3], [4, 5], [6, 7]],
                        ins=[rs_in[c].ap().opt()],
                        outs=[rs_out[c].ap().opt()],
                    )

            # ---- per-(ic, m) attention emitter ----
            def emit_attn(ic, m, den):
                i0 = ic * 512
                nfull = i0 // 128
                if True:
                    e_full = {}  # (h_loc, jbp) -> [128, 1024] (jb pair)
                    e_d1 = {}    # h_loc -> [128, 896]: r=0 (512) | r=1 (384)
                    e_d2 = {}    # h_loc -> [128, 384]: r=2 (256) | r=3 (128)
                    rows_of = (slice(0, 64), slice(64, 128))
                    # full tiles: S^T = K Q^T, exp -> bf16 (no max needed);
                    # 2 j-blocks per psum tile / exp instruction
                    for jbp in range(nfull // 2):
                        pss = [psum.tile([P, 1024], F32, tag="mm2",
                                         name=f"qk{hl}") for hl in range(2)]
                        for half in range(2):
                            jb = 2 * jbp + half
                            hs = slice(half * 512, half * 512 + 512)
                            for h_loc in (0, 1):  # adjacent => row-packed
                                nc.tensor.matmul(
                                    pss[h_loc][:, hs],
                                    lhsT=kT_sb[rows_of[h_loc], m,
                                               jb * 128:(jb + 1) * 128],
                                    rhs=qT_sb[rows_of[h_loc], m, i0:i0 + 512],
                                    start=True, stop=True,
                                )
                        for h_loc in (0, 1):
                            e = epool.tile([P, 1024], BF16, tag="ef2")
                            nc.scalar.activation(e[:], pss[h_loc][:], Exp,
                                                 scale=EXP_SCALE)
                            e_full[(h_loc, jbp)] = e
                    # diagonal region: j-block nfull+r covers i-cols
                    # [r*128, 512) of the chunk in ONE matmul; r in {0,1}
                    # packed into one 2-bank psum (widths 512+384), r in
                    # {2,3} into one bank (256+128); the leading 128 cols
                    # of each r (s==r) get the triangular mask
                    for h_loc in (0, 1):
                        rows = rows_of[h_loc]
                        ps1 = psum.tile([P, 1024], F32, tag="mm2")
                        ps2 = psum.tile([P, 1024], F32, tag="mm2")
                        for r, ps, off in ((0, ps1, 0), (1, ps1, 512),
                                           (2, ps2, 0), (3, ps2, 256)):
                            jb = nfull + r
                            width = (4 - r) * 128
                            nc.tensor.matmul(
                                ps[:, off:off + width],
                                lhsT=kT_sb[rows, m, jb * 128:(jb + 1) * 128],
                                rhs=qT_sb[rows, m, i0 + r * 128:i0 + 512],
                                # off 0 / 512 land at a fresh psum bank: the
                                # first write there must set start (pends
                                # that 2KB zero-region); off 256 reuses r=2's
                                start=(off in (0, 512)), stop=True,
                                skip_group_check=True,
                            )
                        ed1 = edpool.tile([P, 896], BF16, tag="ed1")
                        nc.scalar.activation(ed1[:], ps1[:, 0:896], Exp,
                                             scale=EXP_SCALE)
                        ed2 = edpool.tile([P, 384], BF16, tag="ed2")
                        nc.scalar.activation(ed2[:], ps2[:, 0:384], Exp,
                                             scale=EXP_SCALE)
                        for ed, off in ((ed1, 0), (ed1, 512),
                                        (ed2, 0), (ed2, 256)):
                            nc.vector.tensor_tensor(
                                ed[:, off:off + 128], ed[:, off:off + 128],
                                tri_sb[:], Mult)
                        e_d1[h_loc] = ed1
                        e_d2[h_loc] = ed2
                    # AV: psum[0:64] = unnormalized attn^T, psum[64] = denom
                    for h_loc in (0, 1):
                        h = 2 * m + h_loc
                        vslot = slice(h * 65, (h + 1) * 65)
                        avps = psum_av.tile([P, 512], F32, tag="av")
                        for jbp in range(nfull // 2):
                            ef = e_full[(h_loc, jbp)]
                            for half in range(2):
                                jb = 2 * jbp + half
                                nc.tensor.matmul(
                                    avps[0:65, :],
                                    lhsT=v_sb[:, jb, vslot],
                                    rhs=ef[:, half * 512:half * 512 + 512],
                                    start=(jb == 0), stop=False,
                                    skip_group_check=True,
                                )
                        dslice = {0: (e_d1, 0), 1: (e_d1, 512),
                                  2: (e_d2, 0), 3: (e_d2, 256)}
                        for r in range(4):
                            edd, base = dslice[r]
                            ed = edd[h_loc]
                            width = (4 - r) * 128
                            nc.tensor.matmul(
                                avps[0:65, r * 128:512],
                                lhsT=v_sb[:, nfull + r, vslot],
                                rhs=ed[:, base:base + width],
                                # start=True pends the WHOLE psum bank
                                # (2KB zero-region): only the tile's very
                                # first matmul may set it
                                start=(nfull == 0 and r == 0),
                                stop=(r == 3),
                                skip_group_check=True,
                            )
                        # stash denominator row + unnormalized attn^T
                        # (DVE operands may sit at different partition bases)
                        nc.vector.tensor_copy(
                            den[0:1, h_loc * 512:h_loc * 512 + 512],
                            avps[64:65, :])
                        nc.vector.tensor_copy(
                            attnT_sb[h_loc * 64:h_loc * 64 + 64, m,
                                     i0:i0 + 512],
                            avps[0:64, :])

                # per-(ic, m) softmax normalization: reciprocal of the two
                # denominator rows, DRAM round-trip partition-broadcast,
                # one in-place multiply over both heads
                rec = spool.tile([P, 1024], F32, tag="rec")
                nc.vector.reciprocal_approx_fast(rec[0:1, :], den[0:1, :])
                recb = spool.tile([P, 1024], BF16, tag="recb")
                nc.vector.tensor_copy(recb[0:1, :], rec[0:1, :])
                rd = dpool.tile([1, 1024], BF16, tag="rd")
                nc.sync.dma_start(rd[:], recb[0:1, :])
                rb = spool.tile([P, 512], BF16, tag="rb")
                nc.sync.dma_start(rb[0:64, :],
                                  rd[0:1, 0:512].to_broadcast((64, 512)))
                nc.sync.dma_start(rb[64:128, :],
                                  rd[0:1, 512:1024].to_broadcast((64, 512)))
                nc.vector.tensor_tensor(
                    attnT_sb[:, m, i0:i0 + 512],
                    attnT_sb[:, m, i0:i0 + 512], rb[:], Mult)

            # ---- chunk schedule: interleave projections, attention and
            # deferred chunk-tails so PE always has independent matmuls ----
            emit_vproj(0, min(4, TB))
            for m in range(4):
                emit_qkproj(m)
                den = spool.tile([P, 1024], F32, tag="den")
                emit_attn(0, m, den)
            pending.append(0)
            for ic in range(1, TC):
                emit_vproj(4 * ic, min(4 * ic + 4, TB))
                for m in range(4):
                    den = spool.tile([P, 1024], F32, tag="den")
                    emit_attn(ic, m, den)
                    if m == 1 and pending:
                        # previous chunk's out-proj/RS: emitted mid-attention
                        # so its latency hides behind this chunk's QK/AV
                        emit_chunk_tail(pending.pop(0))
                pending.append(ic)

            while pending:
                emit_chunk_tail(pending.pop(0))

            qrt = T // (2 * NCH)  # rows per core per chunk
            for c in range(NCH):
                for blk in range(max(1, qrt // 128)):
                    rows = min(128, qrt)
                    cv = opool.tile([P, D_MODEL], BF16, tag="cv")
                    nc.gpsimd.dma_start(
                        cv[0:rows, :],
                        rs_out[c].ap()[blk * 128:blk * 128 + rows, :])
                    cf = opool.tile([P, D_MODEL], F32, tag="cf")
                    nc.vector.tensor_copy(cf[0:rows, :], cv[0:rows, :])
                    nc.gpsimd.dma_start(
                        out_d.ap()[c * qrt + blk * 128:
                                   c * qrt + blk * 128 + rows, :],
                        cf[0:rows, :])

            if debug_taps:
                qT_t = nc.dram_tensor("dbg_qT", [P, 4, T], BF16)
                kT_t = nc.dram_tensor("dbg_kT", [P, 4, T], BF16)
                v_t = nc.dram_tensor("dbg_v", [P, TB, H_LOCAL * 65], BF16)
                at_t = nc.dram_tensor("dbg_attnT", [P, 4, T], BF16)
                nc.sync.dma_start(qT_t.ap(), qT_sb[:])
                nc.sync.dma_start(kT_t.ap(), kT_sb[:])
                nc.sync.dma_start(v_t.ap(), v_sb[:])
                nc.sync.dma_start(at_t.ap(), attnT_sb[:])

    nc.finalize()  # Bacc: runs dce/alloc_regs/codegen passes
    return nc


_NC_CACHE = {}


def _get_nc(T):
    if T not in _NC_CACHE:
        _NC_CACHE[T] = build_nc(T)
    return _NC_CACHE[T]


def make_in_maps(x, Wq, Wk, Wv, Wo):
    bf = ml_dtypes.bfloat16
    in_maps = []
    for c in range(N_CORES):
        b, g = divmod(c, 2)
        gs = slice(g * D_LOCAL, (g + 1) * D_LOCAL)
        in_maps.append({
            "xT": np.ascontiguousarray(x[b].T).astype(bf),
            "wqT": np.ascontiguousarray(Wq[gs, :].T).astype(bf),
            "wkT": np.ascontiguousarray(Wk[gs, :].T).astype(bf),
            "wvT": np.ascontiguousarray(Wv[gs, :].T).astype(bf),
            "woT": np.ascontiguousarray(Wo[:, gs].T).astype(bf),
        })
    return in_maps


def assemble_out(outs, B, T, D):
    """Stitch per-core [T//2, D] chunked-RS outputs into [B, T, D]."""
    y = np.empty((B, T, D), np.float32)
    qrt = T // (2 * NCH)
    for b in range(B):
        ev, od = outs[2 * b]["out"], outs[2 * b + 1]["out"]
        for c in range(NCH):
            base = c * (T // NCH)
            y[b, base:base + qrt] = ev[c * qrt:(c + 1) * qrt]
            y[b, base + qrt:base + 2 * qrt] = od[c * qrt:(c + 1) * qrt]
    return y


# test harness hook: set RUN_OPTS["trace"]=True before calling kernel() to
# capture an NTFF profile; the BassKernelResults lands in RUN_OPTS["last"].
RUN_OPTS = {"trace": False, "tmpdir": None, "last": None}


def kernel(x, Wq, Wk, Wv, Wo):
    x = np.asarray(x, dtype=np.float32)
    B, T, D = x.shape
    nc = _get_nc(T)
    in_maps = make_in_maps(np.asarray(x), np.asarray(Wq), np.asarray(Wk),
                           np.asarray(Wv), np.asarray(Wo))
    res = run_bass_kernel_spmd(
        nc, in_maps, core_ids=list(range(N_CORES)),
        trace=RUN_OPTS["trace"], tmpdir=RUN_OPTS["tmpdir"],
    )
    RUN_OPTS["last"] = res
    return assemble_out(res.results, B, T, D)
